# revision 1
# baseline (speedup 1.0000x reference)
"""4-bit quant linear (dense_mlp) on 8 TRN2 NeuronCores — v2.

out[m,o] = sum_i x[m,i] * (scales[o]*q[i,o] - zeros[o]) + bias[o]
         = scales[o] * [ (x @ q)[m,o] + bias[o]/scales[o]
                         + rowsum(x)[m] * (-zeros[o]/scales[o]) ]

Per core (2D shard: tokens 4-way x outfeatures 2-way):
  - Weights kept as RAW int4 values in fp8e4 (exact for 0..15): w_sb is
    8 MB instead of 16 MB bf16, and the scales multiply moves to the
    epilogue (one DVE tensor_tensor by an fp32 scales broadcast).
    Mixed-dtype matmul (bf16 lhsT x fp8 rhs) validated on HW.
  - Unpack per nibble: DVE shift+and (int32) then convert->fp8.
  - x: ScalarE converts fp32->bf16 in two permuted passes (even /
    odd k-nibbles) with accum_out row sums.  The even half is staged
    to DRAM and X-bar DMA-transposed; the odd half is transposed on
    the PE (hybrid — measured fastest).  Freed SBUF double-buffers
    both transpose pools across token groups so group g+1's
    transposes overlap group g's matmuls.
  - Per (128m x 512o) block: 32 accumulating matmuls + one K=2 affine
    matmul (lhsT=[ones;rowsum]^T, rhs=[bias/scales; -zeros/scales])
    adds bias and the zero-point term inside PSUM.  DVE drains with
    the scales multiply fused; DMA out on the SP ring.
"""

import sys

if "/opt/trn_rl_repo" not in sys.path:
    sys.path.insert(0, "/opt/trn_rl_repo")

import numpy as np

import concourse.bass as bass
import concourse.tile as tile
from concourse import bacc, mybir
from concourse.masks import make_identity

B, S, IN, OUT = 4, 2048, 4096, 4096
PACK = 8
M_TOT = B * S
M_SPLIT, O_SPLIT = 4, 2
M_SH, O_SH = M_TOT // M_SPLIT, OUT // O_SPLIT
N_CORES = 8

P = 128
NB = 512  # o-block (one PSUM bank of fp32)
XC = 1024  # x chunk (128 qweight rows * 8 nibbles)
BTG = 4  # token tiles per group
UW = 1024  # unpack o-chunk width
KE = PACK // 2  # even (or odd) nibbles per chunk

FP32 = mybir.dt.float32
BF16 = mybir.dt.bfloat16
FP8 = mybir.dt.float8e4
INT32 = mybir.dt.int32
Alu = mybir.AluOpType
ACT_COPY = mybir.ActivationFunctionType.Copy


def build_kernel(
    m_sh=M_SH,
    o_sh=O_SH,
    in_dim=IN,
    bench_iters=1,
    bench_variant="full",
    xt_grp_bufs=2,  # groups of transpose tiles in flight
    xbar_all=False,  # all 32 k-tiles via X-bar (no PE transposes)
    dve_epilogue=False,  # bias+zeros on DVE instead of PSUM affine matmul
    fp8_kt=8,  # leading k-tiles done in fp8 via DoubleRow (even, %8 in {0,4})
    uw=512,  # unpack o-chunk width (one w tile per chunk)
):
    n_kt = in_dim // P
    n_r = in_dim // XC
    n_bt = m_sh // P
    n_ob = o_sh // NB
    btg = min(BTG, n_bt)
    n_g = n_bt // btg
    n_uw = o_sh // uw

    nc = bacc.Bacc(
        "TRN2", target_bir_lowering=False, debug=False, enable_asserts=False
    )
    x_d = nc.dram_tensor("x", [m_sh, in_dim], FP32, kind="ExternalInput").ap()
    qw_d = nc.dram_tensor(
        "qweight", [in_dim // PACK, o_sh], INT32, kind="ExternalInput"
    ).ap()
    scales_d = nc.dram_tensor("scales", [1, o_sh], FP32, kind="ExternalInput").ap()
    if dve_epilogue:
        epirows_d = nc.dram_tensor(
            "epirows", [2, o_sh], FP32, kind="ExternalInput"
        ).ap()
    else:
        biasnz_d = nc.dram_tensor(
            "biasnz", [2, o_sh], BF16, kind="ExternalInput"
        ).ap()
    out_d = nc.dram_tensor("out", [m_sh, o_sh], FP32, kind="ExternalOutput").ap()

    def bcast_ap(src, parts=P):
        return bass.AP(
            tensor=src.tensor, offset=src.offset, ap=[[0, parts]] + src.ap[1:]
        )

    with tile.TileContext(nc) as tc:
        with (
            tc.tile_pool(name="consts", bufs=1) as consts,
            tc.tile_pool(name="wpool", bufs=1) as wpool,
            tc.tile_pool(name="qwp", bufs=2) as qwp,
            tc.tile_pool(name="nibp", bufs=2) as nibp,
            tc.tile_pool(name="xp", bufs=3) as xp,
            tc.tile_pool(name="xbp", bufs=3) as xbp,
            tc.tile_pool(
                name="xtp",
                bufs=xt_grp_bufs
                * (n_kt if xbar_all else (n_kt - fp8_kt) // 2),
            ) as xtp,
            tc.tile_pool(name="xop", bufs=1 if xbar_all else xt_grp_bufs * btg) as xop,
            tc.tile_pool(name="xfp", bufs=max(1, xt_grp_bufs * btg)) as xfp,
            tc.tile_pool(name="rsp", bufs=2 * btg) as rsp,
            tc.tile_pool(name="outp", bufs=4) as outp,
            tc.tile_pool(name="pst", bufs=2, space="PSUM") as pst,
            tc.tile_pool(name="psm", bufs=5, space="PSUM") as psm,
            tc.tile_pool(name="xbfp", bufs=2, space="DRAM") as xbfp,
        ):
            # ---- constants ----
            identity = consts.tile([P, P], BF16)
            make_identity(nc, identity)
            scales_b = consts.tile([P, o_sh], FP32)
            nc.gpsimd.dma_start(out=scales_b, in_=bcast_ap(scales_d))
            biasnz = bps_b = nzb_b = None
            if dve_epilogue:
                bps_b = consts.tile([P, o_sh], FP32)
                nc.gpsimd.dma_start(out=bps_b, in_=bcast_ap(epirows_d[0:1, :]))
                nzb_b = consts.tile([P, o_sh], FP32)
                nc.gpsimd.dma_start(out=nzb_b, in_=bcast_ap(epirows_d[1:2, :]))
            else:
                biasnz = consts.tile([2, o_sh], BF16)
                nc.gpsimd.dma_start(out=biasnz, in_=biasnz_d)
            dummy = consts.tile([P, 64], FP32)

            # two o-half weight tiles: finer dependency granularity lets
            # the next For_i iteration's unpack start before this one's
            # last matmuls drain
            w_sb = [
                wpool.tile([P, n_kt * uw], FP8, name=f"w{h}")
                for h in range(o_sh // uw)
            ]

            cfg = dict(
                n_kt=n_kt, n_r=n_r, n_bt=n_bt, n_ob=n_ob, btg=btg, n_g=n_g,
                n_uw=n_uw, o_sh=o_sh, uw=uw, variant=bench_variant,
                xbar_all=xbar_all, dve_epilogue=dve_epilogue, fp8_kt=fp8_kt,
            )
            pools = dict(
                qwp=qwp, nibp=nibp, xp=xp, xbp=xbp, xtp=xtp, xop=xop,
                xfp=xfp, rsp=rsp, outp=outp, pst=pst, psm=psm, xbfp=xbfp,
            )
            tens = dict(
                identity=identity, scales_b=scales_b, biasnz=biasnz,
                bps_b=bps_b, nzb_b=nzb_b,
                dummy=dummy, x_d=x_d, qw_d=qw_d, out_d=out_d,
            )
            if bench_iters > 1:
                with tc.For_i(0, bench_iters, 1):
                    _pass_body(nc, pools, cfg, tens, w_sb)
            else:
                _pass_body(nc, pools, cfg, tens, w_sb)
    nc.compile()
    return nc


def _pass_body(nc, pools, cfg, tens, w_sb):
    qwp, nibp, xp, xbp = pools["qwp"], pools["nibp"], pools["xp"], pools["xbp"]
    xtp, xop, rsp, outp = pools["xtp"], pools["xop"], pools["rsp"], pools["outp"]
    pst, psm, xbfp, xfp = pools["pst"], pools["psm"], pools["xbfp"], pools["xfp"]
    n_kt, n_r, n_bt, n_ob = cfg["n_kt"], cfg["n_r"], cfg["n_bt"], cfg["n_ob"]
    btg, n_g, n_uw, o_sh = cfg["btg"], cfg["n_g"], cfg["n_uw"], cfg["o_sh"]
    uw = cfg["uw"]
    variant = cfg["variant"]
    xbar_all, dve_epi = cfg["xbar_all"], cfg["dve_epilogue"]
    fp8_kt = cfg["fp8_kt"]
    identity, scales_b = tens["identity"], tens["scales_b"]
    biasnz, dummy = tens["biasnz"], tens["dummy"]
    bps_b, nzb_b = tens["bps_b"], tens["nzb_b"]
    x_d, qw_d, out_d = tens["x_d"], tens["qw_d"], tens["out_d"]
    in_dim = n_r * XC
    mmonly = variant == "mmonly"

    # ---- weight unpack (o-half-major so PE can start early).  DVE does
    # the bitvec extract; the int->fp8 convert runs on ACT, halving the
    # serial DVE chain that gates the first matmuls ----
    def emit_unpack(ob, conv_on_act=False):
        for r in range(n_r):
            qw_t = qwp.tile([P, uw], INT32, name="qw_t")
            nc.gpsimd.dma_start(
                out=qw_t, in_=qw_d[r * P : (r + 1) * P, bass.ds(ob * uw, uw)]
            )
            for k in range(PACK):
                kp = r * PACK + k
                nib = nibp.tile([P, uw], INT32, name="nib", bufs=3)
                nc.vector.tensor_scalar(
                    nib, qw_t, 4 * k, 0xF,
                    op0=Alu.logical_shift_right, op1=Alu.bitwise_and,
                )
                dst = w_sb[ob][:, bass.ds(kp * uw, uw)]
                if conv_on_act and k % 2 == 0:
                    nc.scalar.copy(out=dst, in_=nib)
                else:
                    nc.vector.tensor_scalar(dst, nib, 1, None, op0=Alu.mult)

    if mmonly:
        nc.vector.memset(w_sb[0][:, 0:XC], 1.0)
        pending_unpack = []
    else:
        n_emit = 1 if n_uw >= 2 else n_uw
        for ob in range(n_emit):
            emit_unpack(ob)
        pending_unpack = list(range(n_emit, n_uw))

    xbfs = [None] * n_g
    lhs2s = [[None] * btg for _ in range(n_g)]
    rss = [[None] * btg for _ in range(n_g)]
    xt_ks_all = [None] * n_g
    xo_all = [[None] * btg for _ in range(n_g)]
    xf_all = [[None] * btg for _ in range(n_g)]
    assert fp8_kt == 0 or (not xbar_all and not mmonly)
    assert fp8_kt % 2 == 0 and (fp8_kt % PACK) % 2 == 0
    # static maps: staging column / xo slot for each bf16 k-tile
    col_of, slot_of = {}, {}
    for _r in range(n_r):
        _ne8 = max(0, min(PACK, fp8_kt - _r * PACK))
        for _e in range(_ne8, PACK):
            _kp = _r * PACK + _e
            if _e % 2 == 0:
                col_of[_kp] = len(col_of)
            else:
                slot_of[_kp] = len(slot_of)
    stage_w = in_dim if xbar_all else len(col_of) * P

    for g in range(n_g):
        if not mmonly:
            # ---- x pipeline: load, permuted converts (+rowsum); staged
            #      half (or all) goes to DRAM for X-bar transposes, the
            #      other half is transposed on the PE ----
            xbf_g = xbfp.tile([btg * P, stage_w], BF16, name="xbf")
            xbfs[g] = xbf_g
            for bi in range(btg):
                bt = g * btg + bi
                bsl = slice(bt * P, (bt + 1) * P)
                if not xbar_all:
                    xo_t = xop.tile(
                        [P, max(1, len(slot_of)) * P], BF16, name="xo"
                    )
                    xo_all[g][bi] = xo_t
                if fp8_kt:
                    xf_t = xfp.tile([P, fp8_kt * P], FP8, name="xf")
                    xf_all[g][bi] = xf_t
                rs_part = rsp.tile([P, 3 * n_r], FP32, name="rs_part")
                for r in range(n_r):
                    x_t = xp.tile([P, XC], FP32, name="x_t")
                    nc.sync.dma_start(
                        out=x_t, in_=x_d[bsl, r * XC : (r + 1) * XC]
                    )
                    x_r = x_t.rearrange("p (j e) -> p e j", e=PACK)
                    ne8 = max(0, min(PACK, fp8_kt - r * PACK))
                    if ne8 > 0:
                        # fp8 range: convert to fp8, upconvert to bf16
                        # (exact; its accum keeps the rowsum consistent
                        # with the fp8 values the matmul uses),
                        # PE-transpose each plane into the fp8 lhsT tile.
                        xf8 = xbp.tile([P, XC], FP8, name="xf8")
                        nc.scalar.activation(
                            xf8[:, : ne8 * P].rearrange(
                                "p (e j) -> p e j", e=ne8
                            ),
                            x_r[:, 0:ne8, :],
                            ACT_COPY, scale=1.0,
                        )
                        xup = xbp.tile([P, XC], BF16, name="xup")
                        nc.vector.tensor_scalar(
                            xup[:, : ne8 * P], xf8[:, : ne8 * P], 1, 0,
                            op0=Alu.mult, op1=Alu.add,
                            accum_out=rs_part[:, 3 * r : 3 * r + 1],
                        )
                        for e in range(ne8):
                            kp = r * PACK + e
                            ps_t = pst.tile([P, P], BF16, name="ps_t")
                            nc.tensor.transpose(
                                ps_t, xup[:, bass.ds(e * P, P)], identity
                            )
                            nc.vector.tensor_scalar(
                                xf_t[:, bass.ds(kp * P, P)],
                                ps_t, 1, None, op0=Alu.mult,
                            )
                    else:
                        nc.gpsimd.memset(rs_part[:, 3 * r : 3 * r + 1], 0.0)
                    if ne8 == PACK:
                        nc.gpsimd.memset(
                            rs_part[:, 3 * r + 1 : 3 * r + 3], 0.0
                        )
                        continue
                    if xbar_all:
                        x_b = xbp.tile([P, XC], BF16, name="x_be")
                        nc.scalar.activation(
                            x_b.rearrange("p (e j) -> p e j", e=PACK),
                            x_r,
                            ACT_COPY, scale=1.0,
                            accum_out=rs_part[:, 2 * r : 2 * r + 1],
                        )
                        nc.scalar.dma_start(
                            out=xbf_g[
                                bi * P : (bi + 1) * P, r * XC : (r + 1) * XC
                            ],
                            in_=x_b,
                        )
                        continue
                    # bf16 even nibbles -> staged for X-bar (+ row sum)
                    n_e = (PACK - ne8) // 2
                    col0 = col_of[r * PACK + ne8]
                    x_be = xbp.tile([P, XC // 2], BF16, name="x_be")
                    nc.scalar.activation(
                        x_be[:, : n_e * P].rearrange(
                            "p (e j) -> p e j", e=n_e
                        ),
                        x_r[:, ne8 : PACK : 2, :],
                        ACT_COPY, scale=1.0,
                        accum_out=rs_part[:, 3 * r + 1 : 3 * r + 2],
                    )
                    nc.scalar.dma_start(
                        out=xbf_g[
                            bi * P : (bi + 1) * P,
                            col0 * P : (col0 + n_e) * P,
                        ],
                        in_=x_be[:, : n_e * P],
                    )
                    # bf16 odd nibbles -> PE transposes (+ row sum).
                    # x_bo tiles live until the PE transposes drain at the
                    # group boundary -> a full group must stay buffered.
                    x_bo = xbp.tile(
                        [P, XC // 2], BF16, name="x_bo",
                        bufs=btg * (n_r - fp8_kt // PACK) + 2,
                    )
                    nc.scalar.activation(
                        x_bo[:, : n_e * P].rearrange(
                            "p (e j) -> p e j", e=n_e
                        ),
                        x_r[:, ne8 + 1 : PACK : 2, :],
                        ACT_COPY, scale=1.0,
                        accum_out=rs_part[:, 3 * r + 2 : 3 * r + 3],
                    )
                    for a in range(n_e):
                        kp = r * PACK + ne8 + 2 * a + 1
                        ps_t = pst.tile([P, P], BF16, name="ps_t")
                        nc.tensor.transpose(
                            ps_t, x_bo[:, bass.ds(a * P, P)], identity
                        )
                        nc.vector.tensor_copy(
                            out=xo_t[:, bass.ds(slot_of[kp] * P, P)],
                            in_=ps_t,
                        )
                rs_t = rsp.tile([P, 1], FP32, name="rs", bufs=n_bt + 2)
                nc.scalar.activation(
                    dummy[:, : 3 * n_r], rs_part, ACT_COPY, scale=1.0,
                    accum_out=rs_t,
                )
                rss[g][bi] = rs_t
                if not dve_epi:
                    # rowsum -> [2,128] bf16 affine lhsT via PE transpose
                    rs2 = rsp.tile([P, 2], BF16, name="rs2", bufs=btg + 2)
                    nc.gpsimd.memset(rs2[:, 0:1], 1.0)
                    nc.vector.tensor_copy(out=rs2[:, 1:2], in_=rs_t)
                    ps_r = pst.tile([2, P], BF16, name="ps_r", bufs=1)
                    nc.tensor.transpose(ps_r, rs2, identity)
                    lhs2 = rsp.tile([2, P], BF16, name="lhs2", bufs=n_bt + 2)
                    nc.vector.tensor_copy(out=lhs2, in_=ps_r)
                    lhs2s[g][bi] = lhs2

            # ---- X-bar transposes for this group's staged k-tiles ----
            xt_ks = {}
            xb_kps = (
                list(range(n_kt)) if xbar_all else sorted(col_of)
            )
            for kp in xb_kps:
                xt_k = xtp.tile([P, btg * P], BF16, name="xt")
                src_col = kp if xbar_all else col_of[kp]
                nc.scalar.dma_start(
                    out=xt_k,
                    in_=xbfs[g][:, src_col * P : (src_col + 1) * P],
                    transpose=True,
                )
                xt_ks[kp] = xt_k
            xt_ks_all[g] = xt_ks
        else:
            if g == 0:
                xt_ks = {}
                for kp in range(0, n_kt, 1 if xbar_all else 2):
                    xt_k = xtp.tile([P, btg * P], BF16, name="xt")
                    nc.gpsimd.memset(xt_k, 0.5)
                    xt_ks[kp] = xt_k
                xo_t = None
                if not xbar_all:
                    xo_t = xop.tile([P, (n_kt // 2) * P], BF16, name="xo")
                    nc.gpsimd.memset(xo_t, 0.5)
                lhs2 = rsp.tile([2, P], BF16, name="lhs2", bufs=n_bt + 2)
                nc.gpsimd.memset(lhs2, 1.0)
                rs_t = rsp.tile([P, 1], FP32, name="rs", bufs=n_bt + 2)
                nc.gpsimd.memset(rs_t, 1.0)
                for gg in range(n_g):
                    xt_ks_all[gg] = xt_ks
                    for bi in range(btg):
                        xo_all[gg][bi] = xo_t
                        lhs2s[gg][bi] = lhs2
                        rss[gg][bi] = rs_t

        def lhs_ap(bi, kp):
            if not xbar_all and kp % 2 == 1:
                return xo_all[g][bi][:, bass.ds(slot_of[kp] * P, P)]
            return xt_ks_all[g][kp][:, bass.ds(bi * P, P)]

        def w_rhs(kp, ob):
            h, obr = divmod(ob, uw // NB)
            return w_sb[h][:, bass.ds(kp * uw + obr * NB, NB)]

        def w_pair_ap(t, ob):
            base = w_rhs(2 * t, ob)
            return bass.AP(
                tensor=base.tensor,
                offset=base.offset,
                ap=[base.ap[0], [uw, 2], base.ap[1]],
            )

        # ---- matmul blocks ----
        for ob in range(n_ob):
            osl = bass.ds(ob * NB, NB)
            for bi in range(btg):
                bt = g * btg + bi
                ps = psm.tile([P, NB], FP32, name="ps")
                # fp8 k-range: DoubleRow over consecutive k-plane pairs
                for t in range(fp8_kt // 2):
                    nc.tensor.matmul(
                        ps,
                        lhsT=xf_all[g][bi][:, bass.ds(2 * t * P, 2 * P)]
                        .rearrange("p (s m) -> p s m", s=2),
                        rhs=w_pair_ap(t, ob),
                        start=(t == 0),
                        stop=False,
                        perf_mode=mybir.MatmulPerfMode.DoubleRow,
                    )
                for kp in range(fp8_kt, n_kt):
                    nc.tensor.matmul(
                        ps,
                        lhsT=lhs_ap(bi, kp),
                        rhs=w_rhs(kp, ob),
                        start=(kp == 0),
                        stop=(kp == n_kt - 1) if dve_epi else False,
                    )
                o_t = outp.tile([P, NB], FP32, name="o_t")
                if dve_epi:
                    # out = scales*(ps + bias/scales) + rowsum*(-zeros)
                    t1 = outp.tile([P, NB], FP32, name="t1", bufs=3)
                    nc.vector.tensor_tensor(
                        t1, ps, bps_b[:, osl], op=Alu.add
                    )
                    nc.vector.tensor_tensor(
                        t1, t1, scales_b[:, osl], op=Alu.mult
                    )
                    nc.vector.scalar_tensor_tensor(
                        o_t, nzb_b[:, osl], rss[g][bi], t1,
                        op0=Alu.mult, op1=Alu.add,
                    )
                else:
                    # += bias/scales + rowsum*(-zeros/scales) inside PSUM
                    nc.tensor.matmul(
                        ps, lhsT=lhs2s[g][bi], rhs=biasnz[:, osl],
                        start=False, stop=True,
                    )
                    nc.vector.tensor_tensor(
                        o_t, ps, scales_b[:, osl], op=Alu.mult
                    )
                nc.sync.dma_start(
                    out=out_d[bt * P : (bt + 1) * P, osl], in_=o_t
                )
            if g == 0 and pending_unpack:
                # one deferred chunk per o-block: keeps DVE drains
                # interleaved with the unpack so PSUM banks recycle
                emit_unpack(pending_unpack.pop(0), conv_on_act=True)


_nc_full = None


def _shard_inputs(x, qweight, scales, zeros, bias):
    import ml_dtypes

    x_flat = np.ascontiguousarray(x.reshape(M_TOT, IN), dtype=np.float32)
    scales_f = np.asarray(scales, dtype=np.float32).reshape(OUT)
    zeros_f = np.asarray(zeros, dtype=np.float32).reshape(OUT)
    bias_f = np.asarray(bias, dtype=np.float32).reshape(OUT)
    biasnz_full = np.stack([bias_f / scales_f, -zeros_f / scales_f]).astype(
        ml_dtypes.bfloat16
    )
    epirows_full = np.stack([bias_f / scales_f, -zeros_f]).astype(np.float32)
    in_maps = []
    for c in range(N_CORES):
        mb_, ob = divmod(c, O_SPLIT)
        osl = slice(ob * O_SH, (ob + 1) * O_SH)
        in_maps.append(
            {
                "x": np.ascontiguousarray(x_flat[mb_ * M_SH : (mb_ + 1) * M_SH]),
                "qweight": np.ascontiguousarray(qweight[:, osl]),
                "scales": np.ascontiguousarray(scales_f[osl][None, :]),
                "biasnz": np.ascontiguousarray(biasnz_full[:, osl]),
                "epirows": np.ascontiguousarray(epirows_full[:, osl]),
            }
        )
    return in_maps


def kernel(x, qweight, scales, zeros, bias):
    global _nc_full
    from concourse import bass_utils

    if _nc_full is None:
        _nc_full = build_kernel()
    in_maps = _shard_inputs(
        np.asarray(x),
        np.asarray(qweight),
        np.asarray(scales),
        np.asarray(zeros),
        np.asarray(bias),
    )
    res = bass_utils.run_bass_kernel_spmd(
        _nc_full, in_maps, core_ids=list(range(N_CORES))
    )
    out = np.empty((M_TOT, OUT), np.float32)
    for c in range(N_CORES):
        mb_, ob = divmod(c, O_SPLIT)
        out[mb_ * M_SH : (mb_ + 1) * M_SH, ob * O_SH : (ob + 1) * O_SH] = res.results[
            c
        ]["out"]
    return out.reshape(B, S, OUT)

